# revision 35
# baseline (speedup 1.0000x reference)
"""Trainium2 Bass kernel for LayerNorm + MLP(16->64->16, ReLU) + residual.

Full inputs in, full output out. Internally: pure data-parallel over 8
NeuronCores (each core gets 16 of the 128 batch rows = 131072 tokens).

Per-core pipeline (token-major chunks of CH tokens, [128, TC, 16] bf16):
  1. LN stats: sq = x*x [DVE bf16 2x], S1/S2 reduce_sum to f32 [DVE];
     smalls: G = 16*S2 - S1^2 = 256*var [DVE], SQv = sqrt(G/256+eps) [ACT],
     R = rstd = recip(SQv) [DVE, bf16], NB = -mu*rstd [DVE, bf16]
  2. y = x*R_bc + NB_bc: two plain tensor_tensor ops on Pool/GPSIMD
     (the HW Pool engine has no tensor_scalar) -- offloads DVE/ACT
  3. DMA xbar transpose -> yT feature-major bf16
  4. mm1: block-diag-2 W1g^T [32,128] @ yT strip -> h psum [PE bf16,
     two NBLK block-chains interleaved to hide relu latency in the
     2-deep PSUM rotation]; relu + b1 in PSUM->SBUF copy [ACT/DVE
     split via relu_dve_mod]
  5. mm2: block-diag-2 W2^T [128,32] @ h -> [128,N] psum; +b2 copy [ACT]
  6. DMA xbar transpose back; residual add [DVE]
  7. DMA out bf16 (host upcasts to f32)

HBM I/O is bf16 (host casts x down, out up); rel-err budget 2e-2 absorbs it.
gamma/beta are folded into W1/b1 on the host (exact algebra):
  h = relu(W1*diag(gamma) @ xn + (b1 + W1@beta)),  xn = (x-mu)*rstd
"""

import sys

sys.path.insert(0, "/opt/trn_rl_repo")

import numpy as np
import ml_dtypes
from contextlib import ExitStack

import concourse.bass as bass
import concourse.bacc as bacc
import concourse.tile as tile
from concourse import mybir
from concourse.bass import ds

F32 = mybir.dt.float32
BF16 = mybir.dt.float16  # full f16 datapath: uniform dtypes (HW NaNs on mixed), more mantissa than bf16
F16 = mybir.dt.float16
AF = mybir.ActivationFunctionType
OP = mybir.AluOpType

N_CORES = 8
H = 16
D = 64
P = 128
EPS = 1e-5

TOK_FULL = 131072  # tokens per core for the real problem
NBLK = 1024        # matmul free-dim block (bf16 moving operand max)


def _bc(ap2, n):
    """[P, T, 1] AP -> [P, T, n] broadcast AP (inner step 0)."""
    return bass.AP(tensor=ap2.tensor, offset=ap2.offset, ap=[*ap2.ap[:2], [0, n]])


def build_nc(tok=TOK_FULL, ch=16384, debug=False, repeat=1, chunks=None,
             relu_dve_mod=3, relu_dve_tail=0, vcopy_act=True,
             y1_pool=True, y2_pool=True, y2_stt=True, resid_split=False,
             bufs_big=8, bufs_small=6, ppb_bufs=4, ppc_bufs=1,
             skew_c=2, skew_d=4, nblk=512):
    """Trace the single-core bass kernel (SPMD across cores).

    relu_dve_mod: every k-th relu block goes to DVE instead of ACT
    (engine balancing); 0 = all on ACT.
    y2_pool/y2_stt: the y*=R2 op on Pool (as stt) vs DVE.
    resid_split: residual add half on DVE, half on Pool.
    """
    if chunks is None:
        assert tok % ch == 0
        chunks = [ch] * (tok // ch)
    assert sum(chunks) == tok
    offs = [sum(chunks[:i]) for i in range(len(chunks))]
    nchunk = len(chunks)
    for c_ in chunks:
        assert c_ % P == 0 and (c_ // P) % 32 == 0

    nc = bacc.Bacc(None, target_bir_lowering=False, debug=debug)
    x_d = nc.dram_tensor("x", [tok, H], BF16, kind="ExternalInput")
    o_d = nc.dram_tensor("out", [tok, H], BF16, kind="ExternalOutput")
    w1_d = nc.dram_tensor("w1r", [P, P], BF16, kind="ExternalInput")
    w2_d = nc.dram_tensor("w2bd", [P, 32], BF16, kind="ExternalInput")
    b1_d = nc.dram_tensor("b1s", [P, 1], F32, kind="ExternalInput")
    b2_d = nc.dram_tensor("b2s", [P, 1], F32, kind="ExternalInput")

    def chunk_view(d, c):
        tc = chunks[c] // P
        return d[offs[c] : offs[c] + chunks[c], :].rearrange(
            "(p t) h -> p t h", p=P, t=tc
        )

    with tile.TileContext(nc) as tc, ExitStack() as ctx:
        consts = ctx.enter_context(tc.tile_pool(name="consts", bufs=1))
        px = ctx.enter_context(tc.tile_pool(name="px", bufs=bufs_big))   # A->D
        pbn = ctx.enter_context(tc.tile_pool(name="pbn", bufs=3))        # A only
        psm = ctx.enter_context(tc.tile_pool(name="psm", bufs=bufs_small))
        py = ctx.enter_context(tc.tile_pool(name="py", bufs=3))          # A only
        pyt = ctx.enter_context(tc.tile_pool(name="pyt", bufs=bufs_small))  # A->C
        ph = ctx.enter_context(tc.tile_pool(name="ph", bufs=3))          # C only
        pvt = ctx.enter_context(tc.tile_pool(name="pvt", bufs=bufs_small))  # C->D
        pvtt = ctx.enter_context(tc.tile_pool(name="pvtt", bufs=3))      # D only
        po = ctx.enter_context(tc.tile_pool(name="po", bufs=3))          # D only
        ppB = ctx.enter_context(tc.tile_pool(name="ppB", bufs=ppb_bufs, space="PSUM"))
        ppC = ctx.enter_context(tc.tile_pool(name="ppC", bufs=ppc_bufs, space="PSUM"))

        w1r = consts.tile([P, P], BF16)  # W1 block-diag replicated 4 strips
        nc.sync.dma_start(w1r[:, :], w1_d[:, :])
        w2s = consts.tile([P, 32], BF16)
        nc.sync.dma_start(w2s[:, :], w2_d[:, :])
        b1s = consts.tile([P, 1], F32)
        nc.sync.dma_start(b1s[:, :], b1_d[:, :])
        b2s = consts.tile([P, 1], F32)
        nc.sync.dma_start(b2s[:, :], b2_d[:, :])
        epsb = consts.tile([P, 1], F32)
        nc.vector.memset(epsb[:, :], EPS)
        zerob = consts.tile([P, 1], F32)
        nc.vector.memset(zerob[:, :], 0.0)
        warm = consts.tile([P, 1], F32)
        nc.scalar.activation(warm, epsb, AF.Sqrt, bias=zerob[:, :], scale=1.0)
        nc.scalar.activation(warm, epsb, AF.Relu, bias=zerob[:, :], scale=1.0)

        def stage_a(c):
            # load + bn_stats + smalls + normalize + transpose to
            # feature-major. Head of the chain at half-chunk granularity to
            # shorten the per-chunk critical path; bn_stats at quarters
            # (hardware free-size cap of 512).
            TC = chunks[c] // P
            NSLAB = TC // 8
            HT = TC // 2
            x_vc = chunk_view(x_d, c)
            xt = px.tile([P, TC, H], BF16, name="xt")
            sq = pbn.tile([P, TC, H], BF16, name="sq")
            # LN stat sums accumulate in f32 (bf16 reduce-adds round at every
            # step on HW: measured 1.9e-2 rel err vs 2.6e-3 with f32 sums).
            # R/NB round once at bf16 write, which is safe.
            S1 = psm.tile([P, TC, 1], F16, name="S1")  # sum(x)
            S2 = psm.tile([P, TC, 1], F16, name="S2")  # sum(x^2)
            P2 = psm.tile([P, TC, 1], F16, name="P2")  # S1^2
            G = psm.tile([P, TC, 1], F16, name="G")    # 16*S2 - S1^2 = 256*var
            SQv = psm.tile([P, TC, 1], F16, name="SQv")  # sqrt(var + eps)
            R = psm.tile([P, TC, 1], BF16, name="R")   # rstd (bf16 for Pool y)
            NB = psm.tile([P, TC, 1], BF16, name="NB")  # -mu*rstd
            with nc.allow_low_precision(reason="f16 sums: ~2e-3 accum error, budget 2e-2"):
                for hh in range(2):
                    hsl = slice(HT * hh, HT * hh + HT)
                    nc.sync.dma_start(xt[:, hsl, :], x_vc[:, hsl, :])
                    nc.vector.tensor_tensor(
                        sq[:, hsl, :], xt[:, hsl, :], xt[:, hsl, :], op=OP.mult
                    )
                    nc.vector.reduce_sum(
                        S1[:, hsl, :], xt[:, hsl, :], axis=mybir.AxisListType.X
                    )
                    nc.vector.reduce_sum(
                        S2[:, hsl, :], sq[:, hsl, :], axis=mybir.AxisListType.X
                    )
            with nc.allow_low_precision(reason="f16 smalls"):
                nc.vector.tensor_tensor(P2[:, :, :], S1[:, :, :], S1[:, :, :], op=OP.mult)
                nc.vector.scalar_tensor_tensor(
                    G[:, :, :], S2[:, :, :], 16.0, P2[:, :, :],
                    op0=OP.mult, op1=OP.subtract
                )
                # sqrt(G/256 + eps) = sqrt(var + eps), so R = recip = rstd
                nc.scalar.activation(SQv[:, :, :], G[:, :, :], AF.Sqrt,
                                     bias=epsb[:, :], scale=1.0 / 256.0)
            with nc.allow_low_precision(reason="uniform-f16 smalls; f16 accum err ~2e-3 vs 2e-2 budget"):
                nc.vector.reciprocal(R[:, :, :], SQv[:, :, :])
                nc.vector.scalar_tensor_tensor(
                    NB[:, :, :], S1[:, :, :], -0.0625, R[:, :, :],
                    op0=OP.mult, op1=OP.mult
                )  # -mu*rstd

            # y = x*rstd - mu*rstd -- two plain tensor_tensor ops so both
            # can run on Pool (the HW Pool engine has no tensor_scalar).
            # Half-chunk granularity so mm1 can start on the first half
            # while the second is still normalizing.
            y = py.tile([P, TC, H], BF16, name="y")
            yT = pyt.tile([P, NSLAB, P], BF16, name="yT")
            for hh in range(2):
                hsl = slice(HT * hh, HT * hh + HT)
                (nc.gpsimd if y1_pool else nc.vector).tensor_tensor(
                    y[:, hsl, :], xt[:, hsl, :], _bc(R[:, hsl, :], H), op=OP.mult
                )
                (nc.gpsimd if y2_pool else nc.vector).tensor_tensor(
                    y[:, hsl, :], y[:, hsl, :], _bc(NB[:, hsl, :], H), op=OP.add
                )
                usl = slice((NSLAB // 2) * hh, (NSLAB // 2) * (hh + 1))
                nc.sync.dma_start_transpose(
                    yT[:, usl, :],
                    y[:, hsl, :].rearrange("p t h -> p (t h)"),
                )
            return xt, yT

        def stage_c(c, yT):
            # mm1 -> relu -> mm2 -> vT. The NB block-chains are interleaved
            # (step order alternates b) so PE feeds chain b+1's mm1 while
            # chain b's relu runs — hides the cross-engine relu latency
            # behind PE work instead of stalling the 2-buf PSUM rotation.
            TC = chunks[c] // P
            NB = (TC * 16) // nblk
            yTf = yT[:, :, :].rearrange("q u c -> q (u c)")
            vT = pvt.tile([P, TC * 16], BF16, name="vT")
            vps = [ppC.tile([P, nblk], F32, name=f"vp{b}") for b in range(NB)]
            for step in range(4 * NB):
                b, s = step % NB, step // NB
                vp = vps[b]
                hp = ppB.tile([P, nblk], F32)
                for e in range(nblk // 512):
                    nc.tensor.matmul(
                        hp[:, ds(512 * e, 512)],
                        w1r[32 * s : 32 * s + 32, :],
                        yTf[32 * s : 32 * s + 32, ds(nblk * b + 512 * e, 512)],
                        start=True,
                        stop=True,
                        tile_position=(32 * s, 0),
                    )
                hs = ph.tile([P, nblk], BF16)
                tail_dve = relu_dve_tail and c >= nchunk - relu_dve_tail
                if tail_dve or (relu_dve_mod and step % relu_dve_mod == 0):
                    nc.vector.tensor_scalar(
                        hs, hp, b1s[:, :], 0.0, op0=OP.add, op1=OP.max
                    )
                else:
                    nc.scalar.activation(
                        hs, hp, AF.Relu, bias=b1s[:, :], scale=1.0
                    )
                for e in range(nblk // 512):
                    nc.tensor.matmul(
                        vp[32 * s : 32 * s + 32, ds(512 * e, 512)],
                        w2s[:, :],
                        hs[:, ds(512 * e, 512)],
                        start=True,
                        stop=True,
                        tile_position=(0, 32 * s),
                    )
                if s == 3:
                    if vcopy_act:
                        nc.scalar.activation(
                            vT[:, ds(nblk * b, nblk)], vp, AF.Identity,
                            bias=b2s[:, :], scale=1.0,
                        )
                    else:
                        nc.vector.tensor_scalar_add(
                            vT[:, ds(nblk * b, nblk)], vp, b2s[:, :]
                        )
            return vT

        def stage_d(c, xt, vT):
            # DMA xbar transpose back + residual + store (bf16 out).
            TC = chunks[c] // P
            HT = TC // 2
            vtt = pvtt.tile([P, TC, H], BF16, name="vtt")
            ot = po.tile([P, TC, H], BF16, name="ot")
            o_vc = chunk_view(o_d, c)
            vtt_v = vtt[:, :, :].rearrange("p t h -> p (t h)").rearrange(
                "p (u c) -> p u c", c=P
            )
            nc.sync.dma_start_transpose(vtt_v, vT[:, :])
            if resid_split:
                for hh in range(2):
                    hsl = slice(HT * hh, HT * hh + HT)
                    eng = nc.vector if hh == 0 else nc.gpsimd
                    eng.tensor_tensor(
                        ot[:, hsl, :], xt[:, hsl, :], vtt[:, hsl, :], op=OP.add
                    )
            else:
                nc.vector.tensor_tensor(ot, xt, vtt, op=OP.add)
            nc.sync.dma_start(o_vc, ot[:, :, :])

        # software-pipelined emission: stage C runs skew_c chunks behind A
        # and D runs skew_d behind, so each engine's in-order stream only
        # sees work whose cross-engine inputs were enabled iterations ago —
        # deep enough that the ~15-hop per-chunk dependency spine stays off
        # the steady-state critical path. Oldest stages first within each
        # iteration.
        live = {}
        for c0 in range((nchunk + skew_d) * repeat):
            c = c0 % (nchunk + skew_d)
            if skew_c <= c and c - skew_c in live:
                xt, yT = live[c - skew_c]
                live[c - skew_c] = (xt, stage_c(c - skew_c, yT))
            if skew_d <= c and c - skew_d in live:
                xt, vT = live.pop(c - skew_d)
                stage_d(c - skew_d, xt, vT)
            if c < nchunk:
                live[c] = stage_a(c)

    return nc


def host_weights(ln_gamma, ln_beta, w1, b1, w2, b2):
    """Fold gamma/beta into W1/b1; build packed block-diag weights."""
    g = np.asarray(ln_gamma, np.float32)
    be = np.asarray(ln_beta, np.float32)
    w1 = np.asarray(w1, np.float32)
    b1 = np.asarray(b1, np.float32)
    w2 = np.asarray(w2, np.float32)
    b2 = np.asarray(b2, np.float32)

    w1gT = (w1 * g[None, :]).T.astype(np.float16)  # [16, 64]
    b1p = (b1 + w1 @ be).astype(np.float32)                # [64]
    w2T = w2.T.astype(np.float16)                  # [64, 16]

    w1bd = np.zeros((32, 128), np.float16)
    w1bd[0:16, 0:64] = w1gT
    w1bd[16:32, 64:128] = w1gT
    w1r = np.tile(w1bd, (4, 1))                            # [128, 128]
    w2bd = np.zeros((128, 32), np.float16)
    w2bd[0:64, 0:16] = w2T
    w2bd[64:128, 16:32] = w2T
    b1s = np.concatenate([b1p, b1p])[:, None].astype(np.float32)   # [128,1]
    b2s = np.tile(b2, 8)[:, None].astype(np.float32)               # [128,1]
    return w1r, w2bd, b1s, b2s


def prep_x(x):
    """Host-side downcast of x to bf16 (halves input DMA)."""
    return np.asarray(x, np.float32).astype(np.float16)


def kernel(x, ln_gamma, ln_beta, w1, b1, w2, b2):
    from concourse.bass_utils import run_bass_kernel_spmd

    x = np.asarray(x, np.float32)
    B, T, Hh = x.shape
    assert (B, T, Hh) == (128, 8192, 16)
    w1r, w2bd, b1s, b2s = host_weights(ln_gamma, ln_beta, w1, b1, w2, b2)

    xs = prep_x(x).reshape(N_CORES, TOK_FULL, H)
    in_maps = [
        {
            "x": np.ascontiguousarray(xs[c]),
            "w1r": w1r,
            "w2bd": w2bd,
            "b1s": b1s,
            "b2s": b2s,
        }
        for c in range(N_CORES)
    ]
    nc = build_nc()
    nc.compile()
    res = run_bass_kernel_spmd(nc, in_maps, core_ids=list(range(N_CORES)))
    out = np.stack([np.asarray(res.results[c]["out"]) for c in range(N_CORES)])
    return out.reshape(B, T, Hh).astype(np.float32)


if __name__ == "__main__":
    nc = build_nc(tok=16384, ch=16384)
    print("traced ok")
